# revision 21
# baseline (speedup 1.0000x reference)
"""ChebGCN (K=2, 2-layer) on 8 Trainium2 NeuronCores.

Full inputs in, full output out. Internally:
  - nodes partitioned by id across 8 cores (graph-parallel, per sharding hint)
  - per-core dest nodes bin-packed into 49 blocks x 128 slots (balanced)
  - messages reduced to post-weight space: tx1@W1 == segsum(norm * (x@W1)[col])
  - separable norm (-dis_r*dis_c) folded host-side: table columns pre-scaled
    by -dis_c, dest scale dis_r folded into the psum eviction copies, x@W0 and
    bias terms pre-divided by dis_r; one-hots are pure 0/1 (single is_equal)
  - tables bf16, gather rows pair-addressed: one 256B descriptor = 2 rows;
    layer-1 source rows freely permuted per core so two edges of one dest
    block share a descriptor (host matching); layer 2 uses natural pairs
  - scatter-add via one-hot matmuls accumulating in fp32 PSUM per dest block
  - layer-2 table exchanged with a bf16 AllGather into Shared dram
Host does sharding prep (matching/pad/index building) and reassembly only.
"""
import sys

for _p in ("/opt/trn_rl_repo",):
    if _p not in sys.path:
        sys.path.insert(0, _p)

import numpy as np
from ml_dtypes import bfloat16
import concourse.bass as bass
import concourse.bacc as bacc
import concourse.mybir as mybir
import concourse.tile as tile
from concourse.bass_utils import run_bass_kernel_spmd

N = 50000
E = 800000
NCORE = 8
SH = 6250           # nodes per core
NB = 49             # dest blocks per core
P = 128
TPC = NB * P        # 6272 table rows per core
TR = NCORE * TPC    # 50176 table rows
F_IN, F_HID, F_OUT = 96, 64, 40
FP = 64             # feature dim of message tables
G = 16              # chunks per dma_gather group
NQ = 2              # SWDGE queues

dt = mybir.dt


# ----------------------------------------------------------------- host prep
def _bin_pack_blocks(deg_local):
    order = np.argsort(-deg_local, kind="stable")
    loads = np.zeros(NB, np.int64)
    counts = np.zeros(NB, np.int32)
    slot = np.full(SH, -1, np.int64)
    big = np.iinfo(np.int64).max
    for l in order:
        b = int(np.argmin(np.where(counts < P, loads, big)))
        slot[l] = b * P + counts[b]
        counts[b] += 1
        loads[b] += deg_local[l]
    return slot


def _wrap_idx(v):
    n = len(v)
    a = np.zeros((16, n // 16), np.int16)
    a[np.arange(n) % 16, np.arange(n) // 16] = v
    return np.tile(a, (8, 1))


def _pack(blk, pidx, half, dp):
    """Group edges into descriptors: edges of one dest block addressing the
    two halves of one table pair share a descriptor. Returns per-descriptor
    arrays (block, pair index, dpA, dpB) -- dp==200 marks an unused half."""
    o = np.lexsort((half, pidx, blk))
    blk, pidx, half, dp = blk[o], pidx[o], half[o], dp[o]
    key = blk.astype(np.int64) * 32768 + pidx
    uk, kid, counts = np.unique(key, return_inverse=True, return_counts=True)
    c0 = np.bincount(kid[half == 0], minlength=len(uk))
    c1 = np.bincount(kid[half == 1], minlength=len(uk))
    mm = np.minimum(c0, c1)
    # rank within (key, half) run
    kh = kid * 2 + half
    _, khid, khc = np.unique(kh, return_inverse=True, return_counts=True)
    srt = np.argsort(khid, kind="stable")
    rank = np.empty(len(kh), np.int64)
    rank[srt] = np.arange(len(kh)) - np.repeat(
        np.cumsum(khc) - khc, khc)
    m0 = (half == 0) & (rank < mm[kid])
    m1 = (half == 1) & (rank < mm[kid])
    single = ~(m0 | m1)
    d_blk = np.concatenate([blk[m0], blk[single]])
    d_pidx = np.concatenate([pidx[m0], pidx[single]])
    d_dpA = np.concatenate([dp[m0], np.where(half[single] == 0, dp[single], 200.0)])
    d_dpB = np.concatenate([dp[m1], np.where(half[single] == 1, dp[single], 200.0)])
    o2 = np.argsort(d_blk, kind="stable")
    return d_blk[o2], d_pidx[o2], d_dpA[o2], d_dpB[o2]


def _pack_arrays(d_blk, d_pidx, d_dpA, d_dpB, C, ucnt):
    """Lay descriptors into [NB, C, 128] column space; emit idx + dp arrays.
    Each block window holds ucnt[b] valid descriptors (real ones first, then
    idx=0 dummies masked by dp=200), then trailing idx=-1 slots the gather
    ucode skips. ucnt is uniform across cores so the runtime count is a
    compile-time immediate."""
    nd = NB * C
    cnt = np.bincount(d_blk, minlength=NB)
    t = np.arange(len(d_blk)) - np.repeat(np.cumsum(cnt) - cnt, cnt)
    colg = d_blk * C + t // P
    p = t % P
    idx = np.full(nd * P, -1, np.int64)
    for b in range(NB):
        w = b * C * P
        idx[w:w + ucnt[b]] = 0
    idx[colg * P + p] = d_pidx
    dpab = np.full((P, nd, 2), 200.0, np.float32)
    dpab[p, colg, 0] = d_dpA
    dpab[p, colg, 1] = d_dpB
    return (_wrap_idx(idx.astype(np.int16)),
            np.ascontiguousarray(dpab.reshape(P, nd * 2).astype(bfloat16)))


def _build_plan(edge_index):
    row = np.asarray(edge_index[0], np.int64)
    col = np.asarray(edge_index[1], np.int64)
    deg = np.bincount(row, minlength=N).astype(np.float32)
    dis = np.where(deg > 0, 1.0 / np.sqrt(np.maximum(deg, 1e-12)), 0.0).astype(np.float32)

    slot_of_node = np.zeros(N, np.int64)
    pi_inv = np.full((NCORE, TPC), -1, np.int64)
    for c in range(NCORE):
        deg_local = deg[c * SH:(c + 1) * SH].astype(np.int64)
        slot = _bin_pack_blocks(deg_local)
        slot_of_node[c * SH:(c + 1) * SH] = slot
        pi_inv[c, slot] = np.arange(c * SH, (c + 1) * SH)

    own = np.arange(N) // SH
    s = slot_of_node
    table_row = own * TPC + (s % P) * NB + (s // P)

    cd = row // SH
    dst_slot = slot_of_node[row]

    # ---- per-core: L1 node pairing (free table) + L2 natural pairing
    cores = []
    C1 = C2 = 1
    for c in range(NCORE):
        m = cd == c
        src = col[m]
        dsl = dst_slot[m]
        blk = (dsl // P).astype(np.int64)
        dp = (dsl % P).astype(np.float64)

        # L1: pair nodes by block-signature so their edges share descriptors
        o = np.lexsort((blk, src))
        s_s, b_s = src[o], blk[o]
        uniq, starts = np.unique(s_s, return_index=True)
        ends = np.append(starts[1:], len(s_s))
        sigs = {}
        for i in range(len(uniq)):
            sig = tuple(b_s[starts[i]:ends[i]].tolist())
            sigs.setdefault(sig, []).append(i)
        order = []
        leftovers = []
        for sig in sorted(sigs):
            nodes = sigs[sig]
            k = len(nodes)
            for j in range(0, k - 1, 2):
                order.append(nodes[j]); order.append(nodes[j + 1])
            if k % 2:
                leftovers.append((sig, nodes[-1]))
        for _, i in sorted(leftovers, key=lambda t: t[0]):
            order.append(i)
        row1_of_uniq = np.empty(len(uniq), np.int64)
        row1_of_uniq[np.array(order)] = np.arange(len(uniq))
        row1 = np.full(N, -1, np.int64)
        row1[uniq] = row1_of_uniq  # table row of each source node

        r1 = row1[src]
        pidx1 = r1 // 2
        half1 = r1 % 2
        db1, dx1, dA1, dB1 = _pack(blk, pidx1, half1, dp)
        C1 = max(C1, int(np.max(np.bincount(db1, minlength=NB) + P - 1) // P))

        # L2: fixed z layout
        sr2 = table_row[src]
        db2, dx2, dA2, dB2 = _pack(blk, sr2 // 2, sr2 % 2, dp)
        C2 = max(C2, int(np.max(np.bincount(db2, minlength=NB) + P - 1) // P))

        cores.append(dict(l1=(db1, dx1, dA1, dB1), l2=(db2, dx2, dA2, dB2),
                          row1=row1, uniq=uniq))

    ucnt1 = np.max([np.bincount(cc["l1"][0], minlength=NB) for cc in cores],
                   axis=0).astype(np.int64)
    ucnt2 = np.max([np.bincount(cc["l2"][0], minlength=NB) for cc in cores],
                   axis=0).astype(np.int64)
    plans = []
    for c in range(NCORE):
        cc = cores[c]
        ix1, dp1 = _pack_arrays(*cc["l1"], C1, ucnt1)
        ix2, dp2 = _pack_arrays(*cc["l2"], C2, ucnt2)
        plans.append(dict(ix1=ix1, dp1=dp1, ix2=ix2, dp2=dp2,
                          row1=cc["row1"], uniq=cc["uniq"]))

    return dict(plans=plans, pi_inv=pi_inv, C1=C1, C2=C2, dis=dis,
                ucnt1=ucnt1, ucnt2=ucnt2, CA=C1, CB=C2)


def _xt_col_of_row(r):
    """Phase A writes table row cn*TPC + p*NB + k from xt column
    cn*TPC + k*P + p; invert to find the xt column of a table row."""
    cn, rr = r // TPC, r % TPC
    p, k = rr // NB, rr % NB
    return cn * TPC + k * P + p


def _build_xt1(x, dis, row1):
    """Per-core L1 table input: column of table row r holds -dis_n * x_n."""
    xt = np.zeros((TR, F_IN), np.float32)
    nodes = np.where(row1 >= 0)[0]
    r = row1[nodes]
    xt[r] = x[nodes] * (-dis[nodes])[:, None]
    out = np.zeros((F_IN, TR), np.float32)
    out[:, _xt_col_of_row(np.arange(TR))] = xt.T
    return np.ascontiguousarray(out.astype(bfloat16))


# ------------------------------------------------------------------ device
def _build_graph(C1, C2, ucnt1, ucnt2, g_sz, nq):
    ND1, ND2 = NB * C1, NB * C2
    nc = bacc.Bacc("TRN2", target_bir_lowering=False, num_devices=NCORE,
                   num_swdge_queues=nq)

    f32, bf16, i16 = dt.float32, dt.bfloat16, dt.int16
    xt_all = nc.dram_tensor("xt_all", [F_IN, TR], bf16, kind="ExternalInput")
    xt_own = nc.dram_tensor("xt_own", [F_IN, TPC], bf16, kind="ExternalInput")
    w10 = nc.dram_tensor("w10", [F_IN, F_HID], bf16, kind="ExternalInput")
    w11 = nc.dram_tensor("w11", [F_IN, F_HID], bf16, kind="ExternalInput")
    w20p = nc.dram_tensor("w20p", [F_HID, FP], bf16, kind="ExternalInput")
    w21p = nc.dram_tensor("w21p", [F_HID, FP], bf16, kind="ExternalInput")
    b1r = nc.dram_tensor("b1r", [1, F_HID], bf16, kind="ExternalInput")
    b2r = nc.dram_tensor("b2r", [1, FP], bf16, kind="ExternalInput")
    invr = nc.dram_tensor("invr", [1, NB * P], bf16, kind="ExternalInput")
    disD = nc.dram_tensor("disD", [P, NB], f32, kind="ExternalInput")
    nd2D = nc.dram_tensor("nd2D", [P, NB], f32, kind="ExternalInput")
    ident = nc.dram_tensor("ident", [P, P], bf16, kind="ExternalInput")
    iota = nc.dram_tensor("iota", [P, P], bf16, kind="ExternalInput")
    ix1 = nc.dram_tensor("ix1", [P, ND1 * 8], i16, kind="ExternalInput")
    dp1 = nc.dram_tensor("dp1", [P, ND1 * 2], bf16, kind="ExternalInput")
    ix2 = nc.dram_tensor("ix2", [P, ND2 * 8], i16, kind="ExternalInput")
    dp2 = nc.dram_tensor("dp2", [P, ND2 * 2], bf16, kind="ExternalInput")
    out = nc.dram_tensor("out", [P, NB, F_OUT], f32, kind="ExternalOutput")

    y1_tab = nc.dram_tensor("y1_tab", [TR, FP], bf16, kind="Internal")
    z_bounce = nc.dram_tensor("z_bounce", [TPC, FP], bf16, kind="Internal")
    z_full = nc.dram_tensor("z_full", [TR, FP], bf16, kind="Internal",
                            addr_space="Shared")

    with tile.TileContext(nc) as tc:
        with (
            tc.tile_pool(name="const", bufs=1) as cpool,
            tc.tile_pool(name="persist", bufs=1) as ppool,
            tc.tile_pool(name="hsp", bufs=2) as hsp,
            tc.tile_pool(name="psT", bufs=2, space="PSUM") as psT,
            tc.tile_pool(name="psZ", bufs=2, space="PSUM") as psZ,
        ):
            def load(pool, src, shape, dtype=bf16, tag=None):
                t = pool.tile(shape, dtype, tag=tag)
                nc.sync.dma_start(t[:], src[:])
                return t

            w10_t = load(cpool, w10, [F_IN, F_HID], tag="w10")
            w11_t = load(cpool, w11, [F_IN, F_HID], tag="w11")
            w20_t = load(cpool, w20p, [F_HID, FP], tag="w20")
            w21_t = load(cpool, w21p, [F_HID, FP], tag="w21")
            b1_t = load(cpool, b1r, [1, F_HID], tag="b1")
            b2_t = load(cpool, b2r, [1, FP], tag="b2")
            inv_t = load(cpool, invr, [1, NB * P], tag="inv")
            disD_t = load(cpool, disD, [P, NB], f32, tag="disD")
            nd2D_t = load(cpool, nd2D, [P, NB], f32, tag="nd2D")
            id_t = load(cpool, ident, [P, P], tag="ident")
            io_t = load(cpool, iota, [P, P], tag="iota")
            ix1_t = load(cpool, ix1, [P, ND1 * 8], i16, tag="ix1")
            dp1_t = load(cpool, dp1, [P, ND1 * 2], tag="dp1")
            ix2_t = load(cpool, ix2, [P, ND2 * 8], i16, tag="ix2")
            dp2_t = load(cpool, dp2, [P, ND2 * 2], tag="dp2")
            xo_t = load(ppool, xt_own, [F_IN, TPC], tag="xown")

            hT = ppool.tile([F_HID, TPC], bf16, tag="hT")
            z_stage = ppool.tile([P, NB, FP], bf16, tag="zst")
            out_stage = ppool.tile([P, NB, F_OUT], f32, tag="ost")

            # ---- phase A: table = (-dis*x) @ W1_1 -> y1_tab (bf16)
            with (
                tc.tile_pool(name="xa2", bufs=3) as xa,
                tc.tile_pool(name="ya2", bufs=2) as ya,
                tc.tile_pool(name="psA", bufs=4, space="PSUM") as psA,
            ):
                BPH = min(25, NB)
                for cn in range(NCORE):
                    yst = ya.tile([P, NB, FP], bf16, tag="yst")
                    b0 = 0
                    for hf, nblk in ((0, BPH), (1, NB - BPH)):
                        if nblk == 0:
                            continue
                        cols = nblk * P
                        xs = xa.tile([F_IN, BPH * P], bf16, tag="xs")
                        nc.sync.dma_start(
                            xs[:, :cols],
                            xt_all[:, cn * TPC + b0 * P: cn * TPC + (b0 + nblk) * P])
                        nb4 = -(-nblk // 4)
                        for q4 in range(nb4):
                            bs = list(range(q4 * 4, min(q4 * 4 + 4, nblk)))
                            ps = psA.tile([P, 4, F_HID], f32, tag="psy")
                            for t, bb in enumerate(bs):
                                nc.tensor.matmul(
                                    out=ps[:, t, :],
                                    lhsT=xs[:, bb * P:(bb + 1) * P],
                                    rhs=w11_t[:], start=True, stop=True)
                            nb_ = len(bs)
                            dstv = yst[:].rearrange("p k f -> p (k f)")[
                                :, (b0 + bs[0]) * F_HID:(b0 + bs[-1] + 1) * F_HID]
                            srcv = ps[:].rearrange("p k f -> p (k f)")[
                                :, :nb_ * F_HID]
                            if q4 % 4 == 3:
                                nc.scalar.copy(dstv, srcv)
                            else:
                                nc.vector.tensor_copy(dstv, srcv)
                        b0 += nblk
                    nc.sync.dma_start(
                        y1_tab[cn * TPC:(cn + 1) * TPC, :].rearrange(
                            "(p k) f -> p k f", p=P),
                        yst[:])

            # ---- spmm pass: one gather per dest block; the runtime count
            # register skips the trailing -1 padding descriptors
            def spmm_pass(tab, evict, sfx, C, ND, ucnt, ix_t, dp_t):
                tabp = tab[:].rearrange("(a two) f -> a (two f)", two=2)
                with (
                    tc.tile_pool(name="mg" + sfx, bufs=8) as mg,
                    tc.tile_pool(name="ohp" + sfx, bufs=6) as ohp,
                    tc.tile_pool(name="psX" + sfx, bufs=4, space="PSUM") as psX,
                ):
                    m_tiles = [None] * NB

                    def gather_block(b):
                        if m_tiles[b] is not None:
                            return m_tiles[b]
                        m = mg.tile([P, C, 2 * FP], bf16, tag="mg")
                        if b < 8:
                            # first pool rotation: zero the buffer so slots the
                            # runtime count skips never expose uninitialized
                            # SBUF (0 * NaN would poison the psum)
                            nc.vector.memset(m[:], 0.0)
                        nc.gpsimd.dma_gather(
                            m[:], tabp,
                            ix_t[:, b * C * 8:(b + 1) * C * 8],
                            C * P, int(ucnt[b]), 2 * FP, single_packet=False,
                            queue_num=b % nq)
                        m_tiles[b] = m
                        return m

                    for b in range(NB):
                        gather_block(b)
                        if b + 1 < NB:
                            gather_block(b + 1)
                        m = m_tiles[b]
                        ps = psX.tile([P, FP], f32, tag="acc")
                        oh = ohp.tile([P, 2 * C * P], bf16, tag="oh")
                        nc.vector.tensor_tensor(
                            out=oh[:].rearrange("p (c j) -> p c j", c=2 * C),
                            in0=dp_t[:, b * 2 * C:(b + 1) * 2 * C].to_broadcast(
                                [P, 2 * C, P]),
                            in1=bass.AP(io_t[:].tensor, io_t[:].offset,
                                        [io_t[:].ap[0], [0, 2 * C], [1, P]]),
                            op=mybir.AluOpType.is_equal)
                        for j in range(C):
                            nc.tensor.matmul(
                                out=ps[:], lhsT=oh[:, (2 * j) * P:(2 * j + 1) * P],
                                rhs=m[:, j, 0:FP],
                                start=(j == 0), stop=False)
                            nc.tensor.matmul(
                                out=ps[:], lhsT=oh[:, (2 * j + 1) * P:(2 * j + 2) * P],
                                rhs=m[:, j, FP:2 * FP],
                                start=False, stop=False)
                        evict(b, ps)

            # ---- layer 1 eviction
            def evict_l1(b, ps):
                nc.tensor.matmul(out=ps[:], lhsT=xo_t[:, b * P:(b + 1) * P],
                                 rhs=w10_t[:], start=False, stop=False)
                nc.tensor.matmul(out=ps[:], lhsT=inv_t[:, b * P:(b + 1) * P],
                                 rhs=b1_t[:], start=False, stop=True)
                hs = hsp.tile([P, F_HID], bf16, tag="hs")
                nc.scalar.activation(hs[:], ps[:], mybir.ActivationFunctionType.Relu)
                pt = psT.tile([F_HID, P], bf16, tag="pt")
                nc.tensor.transpose(out=pt[:], in_=hs[:], identity=id_t[:])
                nc.vector.tensor_copy(hT[:, b * P:(b + 1) * P], pt[:])
                pz = psZ.tile([P, FP], f32, tag="pz")
                nc.tensor.matmul(out=pz[:], lhsT=hT[:, b * P:(b + 1) * P],
                                 rhs=w21_t[:], start=True, stop=True)
                nc.scalar.mul(z_stage[:, b, :], pz[:], nd2D_t[:, b:b + 1])

            spmm_pass(y1_tab, evict_l1, "a", C1, ND1, ucnt1, ix1_t, dp1_t)

            # ---- exchange
            nc.sync.dma_start(
                z_bounce[:].rearrange("(p k) f -> p k f", p=P), z_stage[:])
            nc.gpsimd.collective_compute(
                "AllGather", mybir.AluOpType.bypass,
                replica_groups=[list(range(NCORE))],
                ins=[z_bounce[:].opt()],
                outs=[z_full[:].opt()],
            )

            # ---- layer 2 eviction
            def evict_l2(b, ps):
                nc.tensor.matmul(out=ps[:], lhsT=hT[:, b * P:(b + 1) * P],
                                 rhs=w20_t[:], start=False, stop=False)
                nc.tensor.matmul(out=ps[:], lhsT=inv_t[:, b * P:(b + 1) * P],
                                 rhs=b2_t[:], start=False, stop=True)
                nc.scalar.mul(out_stage[:, b, :], ps[:, :F_OUT], disD_t[:, b:b + 1])

            spmm_pass(z_full, evict_l2, "b", C2, ND2, ucnt2, ix2_t, dp2_t)

            nc.sync.dma_start(out[:], out_stage[:])

    nc.compile()
    return nc


_GRAPH_CACHE = {}


def kernel(x, edge_index, W1_0, W1_1, b1, W2_0, W2_1, b2):
    x = np.asarray(x, np.float32)
    W2_0 = np.asarray(W2_0, np.float32)
    W2_1 = np.asarray(W2_1, np.float32)
    b2 = np.asarray(b2, np.float32)

    plan = _build_plan(edge_index)
    C1, C2 = plan["C1"], plan["C2"]
    dis = plan["dis"]
    pi_inv = plan["pi_inv"]

    w20p = np.zeros((F_HID, FP), np.float32); w20p[:, :F_OUT] = W2_0
    w21p = np.zeros((F_HID, FP), np.float32); w21p[:, :F_OUT] = W2_1
    b2p = np.zeros((1, FP), np.float32); b2p[0, :F_OUT] = b2
    ident = np.eye(P, dtype=np.float32).astype(bfloat16)
    iota = np.tile(np.arange(P, dtype=np.float32), (P, 1)).astype(bfloat16)

    common = dict(
        w10=np.asarray(W1_0, bfloat16), w11=np.asarray(W1_1, bfloat16),
        w20p=w20p.astype(bfloat16), w21p=w21p.astype(bfloat16),
        b1r=np.asarray(b1, np.float32).reshape(1, F_HID).astype(bfloat16),
        b2r=b2p.astype(bfloat16),
        ident=ident, iota=iota,
    )
    in_maps = []
    for c in range(NCORE):
        pl = plan["plans"][c]
        # per-slot dest scales (slot-ordered): node at slot s of core c
        nodes = pi_inv[c]
        valid = nodes >= 0
        dslot = np.zeros(TPC, np.float32)
        dslot[valid] = dis[nodes[valid]]
        sD = np.where(dslot > 0, dslot, 1.0)       # s_r (=dis or 1)
        invD = np.zeros(TPC, np.float32)
        invD[valid] = 1.0 / sD[valid]              # 1/s_r (0 on empty slots)
        nd2 = -dslot * dslot                        # -dis^2 (z-table scale)
        # slot s = b*P + p maps to [p, b] tiles
        disD = sD.reshape(NB, P).T.copy()
        nd2D = nd2.reshape(NB, P).T.copy()
        # xt_own columns: x_n / s_r, slot-major
        xo = np.zeros((TPC, F_IN), np.float32)
        xo[valid] = x[nodes[valid]] * invD[valid][:, None]
        m = dict(common)
        m["xt_all"] = _build_xt1(x, dis, pl["row1"])
        m["xt_own"] = np.ascontiguousarray(xo.T.astype(bfloat16))
        m["invr"] = invD.reshape(1, TPC).astype(bfloat16)
        m["disD"] = disD.astype(np.float32)
        m["nd2D"] = nd2D.astype(np.float32)
        m["ix1"] = pl["ix1"]; m["dp1"] = pl["dp1"]
        m["ix2"] = pl["ix2"]; m["dp2"] = pl["dp2"]
        in_maps.append(m)

    res = None
    last_exc = None
    u1 = tuple(int(v) for v in plan["ucnt1"])
    u2 = tuple(int(v) for v in plan["ucnt2"])
    for g_try, nq_try in ((G, NQ), (16, 2), (4, 1)):
        key = (C1, C2, u1, u2, g_try, nq_try)
        try:
            if key not in _GRAPH_CACHE:
                _GRAPH_CACHE[key] = _build_graph(C1, C2, u1, u2, g_try, nq_try)
            res = run_bass_kernel_spmd(
                _GRAPH_CACHE[key], in_maps, core_ids=list(range(NCORE)))
            break
        except Exception as e:  # noqa: BLE001 - retry with safer gather config
            last_exc = e
            import time as _t
            _t.sleep(5)
    if res is None:
        raise last_exc
    kernel.last_result = res

    out_full = np.zeros((N, F_OUT), np.float32)
    for c in range(NCORE):
        o = res.results[c]["out"].transpose(1, 0, 2).reshape(TPC, F_OUT)
        valid = pi_inv[c] >= 0
        out_full[pi_inv[c][valid]] = o[valid]
    return out_full


# revision 23
# speedup vs baseline: 1.5273x; 1.5273x over previous
"""ChebGCN (K=2, 2-layer) on 8 Trainium2 NeuronCores.

Full inputs in, full output out. Internally:
  - nodes partitioned by id across 8 cores (graph-parallel, per sharding hint)
  - per-core dest nodes bin-packed into 49 blocks x 128 slots (balanced)
  - messages reduced to post-weight space: tx1@W1 == segsum(norm * (x@W1)[col])
  - separable norm (-dis_r*dis_c) folded host-side: table columns pre-scaled
    by -dis_c, dest scale dis_r folded into the psum eviction copies, x@W0 and
    bias terms pre-divided by dis_r; one-hots are pure 0/1 (single is_equal)
  - tables bf16, gather rows pair-addressed: one 256B descriptor = 2 rows;
    layer-1 source rows freely permuted per core so two edges of one dest
    block share a descriptor (host matching); layer 2 uses natural pairs
  - scatter-add via one-hot matmuls accumulating in fp32 PSUM per dest block
  - layer-2 table exchanged with a bf16 AllGather into Shared dram
Host does sharding prep (matching/pad/index building) and reassembly only.
"""
import sys

for _p in ("/opt/trn_rl_repo",):
    if _p not in sys.path:
        sys.path.insert(0, _p)

import numpy as np
from ml_dtypes import bfloat16
import concourse.bass as bass
import concourse.bacc as bacc
import concourse.mybir as mybir
import concourse.tile as tile
from concourse.bass_utils import run_bass_kernel_spmd

N = 50000
E = 800000
NCORE = 8
SH = 6250           # nodes per core
NB = 49             # dest blocks per core
P = 128
TPC = NB * P        # 6272 table rows per core
TR = NCORE * TPC    # 50176 table rows
F_IN, F_HID, F_OUT = 96, 64, 40
FP = 64             # feature dim of message tables
G = 16              # chunks per dma_gather group
NQ = 4              # SWDGE queues

dt = mybir.dt


# ----------------------------------------------------------------- host prep
def _bin_pack_blocks(deg_local):
    order = np.argsort(-deg_local, kind="stable")
    loads = np.zeros(NB, np.int64)
    counts = np.zeros(NB, np.int32)
    slot = np.full(SH, -1, np.int64)
    big = np.iinfo(np.int64).max
    for l in order:
        b = int(np.argmin(np.where(counts < P, loads, big)))
        slot[l] = b * P + counts[b]
        counts[b] += 1
        loads[b] += deg_local[l]
    return slot


def _wrap_idx(v):
    n = len(v)
    a = np.zeros((16, n // 16), np.int16)
    a[np.arange(n) % 16, np.arange(n) // 16] = v
    return np.tile(a, (8, 1))


def _pack(blk, pidx, half, dp):
    """Group edges into descriptors: edges of one dest block addressing the
    two halves of one table pair share a descriptor. Returns per-descriptor
    arrays (block, pair index, dpA, dpB) -- dp==200 marks an unused half."""
    o = np.lexsort((half, pidx, blk))
    blk, pidx, half, dp = blk[o], pidx[o], half[o], dp[o]
    key = blk.astype(np.int64) * 32768 + pidx
    uk, kid, counts = np.unique(key, return_inverse=True, return_counts=True)
    c0 = np.bincount(kid[half == 0], minlength=len(uk))
    c1 = np.bincount(kid[half == 1], minlength=len(uk))
    mm = np.minimum(c0, c1)
    # rank within (key, half) run
    kh = kid * 2 + half
    _, khid, khc = np.unique(kh, return_inverse=True, return_counts=True)
    srt = np.argsort(khid, kind="stable")
    rank = np.empty(len(kh), np.int64)
    rank[srt] = np.arange(len(kh)) - np.repeat(
        np.cumsum(khc) - khc, khc)
    m0 = (half == 0) & (rank < mm[kid])
    m1 = (half == 1) & (rank < mm[kid])
    single = ~(m0 | m1)
    d_blk = np.concatenate([blk[m0], blk[single]])
    d_pidx = np.concatenate([pidx[m0], pidx[single]])
    d_dpA = np.concatenate([dp[m0], np.where(half[single] == 0, dp[single], 200.0)])
    d_dpB = np.concatenate([dp[m1], np.where(half[single] == 1, dp[single], 200.0)])
    o2 = np.argsort(d_blk, kind="stable")
    return d_blk[o2], d_pidx[o2], d_dpA[o2], d_dpB[o2]


def _pack_arrays(d_blk, d_pidx, d_dpA, d_dpB, C, ucnt):
    """Lay descriptors into [NB, C, 128] column space; emit idx + dp arrays.
    Each block window holds ucnt[b] valid descriptors (real ones first, then
    idx=0 dummies masked by dp=200), then trailing idx=-1 slots the gather
    ucode skips. ucnt is uniform across cores so the runtime count is a
    compile-time immediate."""
    nd = NB * C
    cnt = np.bincount(d_blk, minlength=NB)
    t = np.arange(len(d_blk)) - np.repeat(np.cumsum(cnt) - cnt, cnt)
    colg = d_blk * C + t // P
    p = t % P
    idx = np.full(nd * P, -1, np.int64)
    for b in range(NB):
        w = b * C * P
        idx[w:w + ucnt[b]] = 0
    idx[colg * P + p] = d_pidx
    dpab = np.full((P, nd, 2), 200.0, np.float32)
    dpab[p, colg, 0] = d_dpA
    dpab[p, colg, 1] = d_dpB
    return (_wrap_idx(idx.astype(np.int16)),
            np.ascontiguousarray(dpab.reshape(P, nd * 2).astype(bfloat16)))


def _build_plan(edge_index):
    row = np.asarray(edge_index[0], np.int64)
    col = np.asarray(edge_index[1], np.int64)
    deg = np.bincount(row, minlength=N).astype(np.float32)
    dis = np.where(deg > 0, 1.0 / np.sqrt(np.maximum(deg, 1e-12)), 0.0).astype(np.float32)

    slot_of_node = np.zeros(N, np.int64)
    pi_inv = np.full((NCORE, TPC), -1, np.int64)
    for c in range(NCORE):
        deg_local = deg[c * SH:(c + 1) * SH].astype(np.int64)
        slot = _bin_pack_blocks(deg_local)
        slot_of_node[c * SH:(c + 1) * SH] = slot
        pi_inv[c, slot] = np.arange(c * SH, (c + 1) * SH)

    own = np.arange(N) // SH
    s = slot_of_node
    table_row = own * TPC + (s % P) * NB + (s // P)

    cd = row // SH
    dst_slot = slot_of_node[row]

    # ---- per-core: L1 node pairing (free table) + L2 natural pairing
    cores = []
    C1 = C2 = 1
    for c in range(NCORE):
        m = cd == c
        src = col[m]
        dsl = dst_slot[m]
        blk = (dsl // P).astype(np.int64)
        dp = (dsl % P).astype(np.float64)

        # L1: pair nodes by block-signature so their edges share descriptors
        o = np.lexsort((blk, src))
        s_s, b_s = src[o], blk[o]
        uniq, starts = np.unique(s_s, return_index=True)
        ends = np.append(starts[1:], len(s_s))
        sigs = {}
        for i in range(len(uniq)):
            sig = tuple(b_s[starts[i]:ends[i]].tolist())
            sigs.setdefault(sig, []).append(i)
        order = []
        leftovers = []
        for sig in sorted(sigs):
            nodes = sigs[sig]
            k = len(nodes)
            for j in range(0, k - 1, 2):
                order.append(nodes[j]); order.append(nodes[j + 1])
            if k % 2:
                leftovers.append((sig, nodes[-1]))
        for _, i in sorted(leftovers, key=lambda t: t[0]):
            order.append(i)
        # scatter pair-slots over the full table with a stride permutation so
        # each block's gather addresses spread across DRAM channels (the
        # signature sort would otherwise cluster a block's rows together)
        j = np.arange(len(uniq))
        slot_perm = (j // 2 * 10007) % (TR // 2)
        row1_of_uniq = np.empty(len(uniq), np.int64)
        row1_of_uniq[np.array(order)] = 2 * slot_perm + (j % 2)
        row1 = np.full(N, -1, np.int64)
        row1[uniq] = row1_of_uniq  # table row of each source node

        r1 = row1[src]
        pidx1 = r1 // 2
        half1 = r1 % 2
        db1, dx1, dA1, dB1 = _pack(blk, pidx1, half1, dp)
        C1 = max(C1, int(np.max(np.bincount(db1, minlength=NB) + P - 1) // P))

        # L2: fixed z layout
        sr2 = table_row[src]
        db2, dx2, dA2, dB2 = _pack(blk, sr2 // 2, sr2 % 2, dp)
        C2 = max(C2, int(np.max(np.bincount(db2, minlength=NB) + P - 1) // P))

        cores.append(dict(l1=(db1, dx1, dA1, dB1), l2=(db2, dx2, dA2, dB2),
                          row1=row1, uniq=uniq))

    ucnt1 = np.max([np.bincount(cc["l1"][0], minlength=NB) for cc in cores],
                   axis=0).astype(np.int64)
    ucnt2 = np.max([np.bincount(cc["l2"][0], minlength=NB) for cc in cores],
                   axis=0).astype(np.int64)
    plans = []
    for c in range(NCORE):
        cc = cores[c]
        ix1, dp1 = _pack_arrays(*cc["l1"], C1, ucnt1)
        ix2, dp2 = _pack_arrays(*cc["l2"], C2, ucnt2)
        plans.append(dict(ix1=ix1, dp1=dp1, ix2=ix2, dp2=dp2,
                          row1=cc["row1"], uniq=cc["uniq"]))

    return dict(plans=plans, pi_inv=pi_inv, C1=C1, C2=C2, dis=dis,
                ucnt1=ucnt1, ucnt2=ucnt2, CA=C1, CB=C2)


def _xt_col_of_row(r):
    """Phase A writes table row cn*TPC + p*NB + k from xt column
    cn*TPC + k*P + p; invert to find the xt column of a table row."""
    cn, rr = r // TPC, r % TPC
    p, k = rr // NB, rr % NB
    return cn * TPC + k * P + p


def _build_xt1(x, dis, row1):
    """Per-core L1 table input: column of table row r holds -dis_n * x_n."""
    xt = np.zeros((TR, F_IN), np.float32)
    nodes = np.where(row1 >= 0)[0]
    r = row1[nodes]
    xt[r] = x[nodes] * (-dis[nodes])[:, None]
    out = np.zeros((F_IN, TR), np.float32)
    out[:, _xt_col_of_row(np.arange(TR))] = xt.T
    return np.ascontiguousarray(out.astype(bfloat16))


# ------------------------------------------------------------------ device
def _build_graph(C1, C2, ucnt1, ucnt2, g_sz, nq):
    ND1, ND2 = NB * C1, NB * C2
    nc = bacc.Bacc("TRN2", target_bir_lowering=False, num_devices=NCORE,
                   num_swdge_queues=nq)

    f32, bf16, i16 = dt.float32, dt.bfloat16, dt.int16
    xt_all = nc.dram_tensor("xt_all", [F_IN, TR], bf16, kind="ExternalInput")
    xt_own = nc.dram_tensor("xt_own", [F_IN, TPC], bf16, kind="ExternalInput")
    w10 = nc.dram_tensor("w10", [F_IN, F_HID], bf16, kind="ExternalInput")
    w11 = nc.dram_tensor("w11", [F_IN, F_HID], bf16, kind="ExternalInput")
    w20p = nc.dram_tensor("w20p", [F_HID, FP], bf16, kind="ExternalInput")
    w21p = nc.dram_tensor("w21p", [F_HID, FP], bf16, kind="ExternalInput")
    b1r = nc.dram_tensor("b1r", [1, F_HID], bf16, kind="ExternalInput")
    b2r = nc.dram_tensor("b2r", [1, FP], bf16, kind="ExternalInput")
    invr = nc.dram_tensor("invr", [1, NB * P], bf16, kind="ExternalInput")
    disD = nc.dram_tensor("disD", [P, NB], f32, kind="ExternalInput")
    nd2D = nc.dram_tensor("nd2D", [P, NB], f32, kind="ExternalInput")
    ident = nc.dram_tensor("ident", [P, P], bf16, kind="ExternalInput")
    iota = nc.dram_tensor("iota", [P, P], bf16, kind="ExternalInput")
    ix1 = nc.dram_tensor("ix1", [P, ND1 * 8], i16, kind="ExternalInput")
    dp1 = nc.dram_tensor("dp1", [P, ND1 * 2], bf16, kind="ExternalInput")
    ix2 = nc.dram_tensor("ix2", [P, ND2 * 8], i16, kind="ExternalInput")
    dp2 = nc.dram_tensor("dp2", [P, ND2 * 2], bf16, kind="ExternalInput")
    out = nc.dram_tensor("out", [P, NB, F_OUT], f32, kind="ExternalOutput")

    y1_tab = nc.dram_tensor("y1_tab", [TR, FP], bf16, kind="Internal")
    z_bounce = nc.dram_tensor("z_bounce", [TPC, FP], bf16, kind="Internal")
    z_full = nc.dram_tensor("z_full", [TR, FP], bf16, kind="Internal",
                            addr_space="Shared")

    with tile.TileContext(nc) as tc:
        with (
            tc.tile_pool(name="const", bufs=1) as cpool,
            tc.tile_pool(name="persist", bufs=1) as ppool,
            tc.tile_pool(name="hsp", bufs=2) as hsp,
            tc.tile_pool(name="psT", bufs=2, space="PSUM") as psT,
            tc.tile_pool(name="psZ", bufs=2, space="PSUM") as psZ,
        ):
            def load(pool, src, shape, dtype=bf16, tag=None):
                t = pool.tile(shape, dtype, tag=tag)
                nc.sync.dma_start(t[:], src[:])
                return t

            w10_t = load(cpool, w10, [F_IN, F_HID], tag="w10")
            w11_t = load(cpool, w11, [F_IN, F_HID], tag="w11")
            w20_t = load(cpool, w20p, [F_HID, FP], tag="w20")
            w21_t = load(cpool, w21p, [F_HID, FP], tag="w21")
            b1_t = load(cpool, b1r, [1, F_HID], tag="b1")
            b2_t = load(cpool, b2r, [1, FP], tag="b2")
            inv_t = load(cpool, invr, [1, NB * P], tag="inv")
            disD_t = load(cpool, disD, [P, NB], f32, tag="disD")
            nd2D_t = load(cpool, nd2D, [P, NB], f32, tag="nd2D")
            id_t = load(cpool, ident, [P, P], tag="ident")
            io_t = load(cpool, iota, [P, P], tag="iota")
            ix1_t = load(cpool, ix1, [P, ND1 * 8], i16, tag="ix1")
            dp1_t = load(cpool, dp1, [P, ND1 * 2], tag="dp1")
            ix2_t = load(cpool, ix2, [P, ND2 * 8], i16, tag="ix2")
            dp2_t = load(cpool, dp2, [P, ND2 * 2], tag="dp2")
            xo_t = load(ppool, xt_own, [F_IN, TPC], tag="xown")

            hT = ppool.tile([F_HID, TPC], bf16, tag="hT")
            z_stage = ppool.tile([P, NB, FP], bf16, tag="zst")
            out_stage = ppool.tile([P, NB, F_OUT], f32, tag="ost")

            # ---- phase A: table = (-dis*x) @ W1_1 -> y1_tab (bf16)
            with (
                tc.tile_pool(name="xa2", bufs=3) as xa,
                tc.tile_pool(name="ya2", bufs=2) as ya,
                tc.tile_pool(name="psA", bufs=4, space="PSUM") as psA,
            ):
                BPH = min(25, NB)
                for cn in range(NCORE):
                    yst = ya.tile([P, NB, FP], bf16, tag="yst")
                    b0 = 0
                    for hf, nblk in ((0, BPH), (1, NB - BPH)):
                        if nblk == 0:
                            continue
                        cols = nblk * P
                        xs = xa.tile([F_IN, BPH * P], bf16, tag="xs")
                        nc.sync.dma_start(
                            xs[:, :cols],
                            xt_all[:, cn * TPC + b0 * P: cn * TPC + (b0 + nblk) * P])
                        nb4 = -(-nblk // 4)
                        for q4 in range(nb4):
                            bs = list(range(q4 * 4, min(q4 * 4 + 4, nblk)))
                            ps = psA.tile([P, 4, F_HID], f32, tag="psy")
                            for t, bb in enumerate(bs):
                                nc.tensor.matmul(
                                    out=ps[:, t, :],
                                    lhsT=xs[:, bb * P:(bb + 1) * P],
                                    rhs=w11_t[:], start=True, stop=True)
                            nb_ = len(bs)
                            dstv = yst[:].rearrange("p k f -> p (k f)")[
                                :, (b0 + bs[0]) * F_HID:(b0 + bs[-1] + 1) * F_HID]
                            srcv = ps[:].rearrange("p k f -> p (k f)")[
                                :, :nb_ * F_HID]
                            if q4 % 4 == 3:
                                nc.scalar.copy(dstv, srcv)
                            else:
                                nc.vector.tensor_copy(dstv, srcv)
                        b0 += nblk
                    nc.sync.dma_start(
                        y1_tab[cn * TPC:(cn + 1) * TPC, :].rearrange(
                            "(p k) f -> p k f", p=P),
                        yst[:])

            # ---- spmm pass: one gather per dest block; the runtime count
            # register skips the trailing -1 padding descriptors
            def spmm_pass(tab, evict, sfx, C, ND, ucnt, ix_t, dp_t):
                tabp = tab[:].rearrange("(a two) f -> a (two f)", two=2)
                with (
                    tc.tile_pool(name="mg" + sfx, bufs=8) as mg,
                    tc.tile_pool(name="ohp" + sfx, bufs=6) as ohp,
                    tc.tile_pool(name="psX" + sfx, bufs=4, space="PSUM") as psX,
                ):
                    m_tiles = [None] * NB

                    def gather_block(b):
                        if m_tiles[b] is not None:
                            return m_tiles[b]
                        m = mg.tile([P, C, 2 * FP], bf16, tag="mg")
                        if b < 8:
                            # first pool rotation: zero the buffer so slots the
                            # runtime count skips never expose uninitialized
                            # SBUF (0 * NaN would poison the psum)
                            nc.vector.memset(m[:], 0.0)
                        nc.gpsimd.dma_gather(
                            m[:], tabp,
                            ix_t[:, b * C * 8:(b + 1) * C * 8],
                            C * P, int(ucnt[b]), 2 * FP, single_packet=False,
                            queue_num=b % nq)
                        m_tiles[b] = m
                        return m

                    for b in range(NB):
                        gather_block(b)
                        if b + 1 < NB:
                            gather_block(b + 1)
                        m = m_tiles[b]
                        ps = psX.tile([P, FP], f32, tag="acc")
                        oh = ohp.tile([P, 2 * C * P], bf16, tag="oh")
                        nc.vector.tensor_tensor(
                            out=oh[:].rearrange("p (c j) -> p c j", c=2 * C),
                            in0=dp_t[:, b * 2 * C:(b + 1) * 2 * C].to_broadcast(
                                [P, 2 * C, P]),
                            in1=bass.AP(io_t[:].tensor, io_t[:].offset,
                                        [io_t[:].ap[0], [0, 2 * C], [1, P]]),
                            op=mybir.AluOpType.is_equal)
                        for j in range(C):
                            nc.tensor.matmul(
                                out=ps[:], lhsT=oh[:, (2 * j) * P:(2 * j + 1) * P],
                                rhs=m[:, j, 0:FP],
                                start=(j == 0), stop=False)
                            nc.tensor.matmul(
                                out=ps[:], lhsT=oh[:, (2 * j + 1) * P:(2 * j + 2) * P],
                                rhs=m[:, j, FP:2 * FP],
                                start=False, stop=False)
                        evict(b, ps)

            # ---- layer 1 eviction
            def evict_l1(b, ps):
                nc.tensor.matmul(out=ps[:], lhsT=xo_t[:, b * P:(b + 1) * P],
                                 rhs=w10_t[:], start=False, stop=False)
                nc.tensor.matmul(out=ps[:], lhsT=inv_t[:, b * P:(b + 1) * P],
                                 rhs=b1_t[:], start=False, stop=True)
                hs = hsp.tile([P, F_HID], bf16, tag="hs")
                nc.scalar.activation(hs[:], ps[:], mybir.ActivationFunctionType.Relu)
                pt = psT.tile([F_HID, P], bf16, tag="pt")
                nc.tensor.transpose(out=pt[:], in_=hs[:], identity=id_t[:])
                nc.vector.tensor_copy(hT[:, b * P:(b + 1) * P], pt[:])
                pz = psZ.tile([P, FP], f32, tag="pz")
                nc.tensor.matmul(out=pz[:], lhsT=hT[:, b * P:(b + 1) * P],
                                 rhs=w21_t[:], start=True, stop=True)
                nc.scalar.mul(z_stage[:, b, :], pz[:], nd2D_t[:, b:b + 1])

            spmm_pass(y1_tab, evict_l1, "a", C1, ND1, ucnt1, ix1_t, dp1_t)

            # ---- exchange
            nc.sync.dma_start(
                z_bounce[:].rearrange("(p k) f -> p k f", p=P), z_stage[:])
            nc.gpsimd.collective_compute(
                "AllGather", mybir.AluOpType.bypass,
                replica_groups=[list(range(NCORE))],
                ins=[z_bounce[:].opt()],
                outs=[z_full[:].opt()],
            )

            # ---- layer 2 eviction
            def evict_l2(b, ps):
                nc.tensor.matmul(out=ps[:], lhsT=hT[:, b * P:(b + 1) * P],
                                 rhs=w20_t[:], start=False, stop=False)
                nc.tensor.matmul(out=ps[:], lhsT=inv_t[:, b * P:(b + 1) * P],
                                 rhs=b2_t[:], start=False, stop=True)
                nc.scalar.mul(out_stage[:, b, :], ps[:, :F_OUT], disD_t[:, b:b + 1])

            spmm_pass(z_full, evict_l2, "b", C2, ND2, ucnt2, ix2_t, dp2_t)

            nc.sync.dma_start(out[:], out_stage[:])

    nc.compile()
    return nc


_GRAPH_CACHE = {}


def kernel(x, edge_index, W1_0, W1_1, b1, W2_0, W2_1, b2):
    x = np.asarray(x, np.float32)
    W2_0 = np.asarray(W2_0, np.float32)
    W2_1 = np.asarray(W2_1, np.float32)
    b2 = np.asarray(b2, np.float32)

    plan = _build_plan(edge_index)
    C1, C2 = plan["C1"], plan["C2"]
    dis = plan["dis"]
    pi_inv = plan["pi_inv"]

    w20p = np.zeros((F_HID, FP), np.float32); w20p[:, :F_OUT] = W2_0
    w21p = np.zeros((F_HID, FP), np.float32); w21p[:, :F_OUT] = W2_1
    b2p = np.zeros((1, FP), np.float32); b2p[0, :F_OUT] = b2
    ident = np.eye(P, dtype=np.float32).astype(bfloat16)
    iota = np.tile(np.arange(P, dtype=np.float32), (P, 1)).astype(bfloat16)

    common = dict(
        w10=np.asarray(W1_0, bfloat16), w11=np.asarray(W1_1, bfloat16),
        w20p=w20p.astype(bfloat16), w21p=w21p.astype(bfloat16),
        b1r=np.asarray(b1, np.float32).reshape(1, F_HID).astype(bfloat16),
        b2r=b2p.astype(bfloat16),
        ident=ident, iota=iota,
    )
    in_maps = []
    for c in range(NCORE):
        pl = plan["plans"][c]
        # per-slot dest scales (slot-ordered): node at slot s of core c
        nodes = pi_inv[c]
        valid = nodes >= 0
        dslot = np.zeros(TPC, np.float32)
        dslot[valid] = dis[nodes[valid]]
        sD = np.where(dslot > 0, dslot, 1.0)       # s_r (=dis or 1)
        invD = np.zeros(TPC, np.float32)
        invD[valid] = 1.0 / sD[valid]              # 1/s_r (0 on empty slots)
        nd2 = -dslot * dslot                        # -dis^2 (z-table scale)
        # slot s = b*P + p maps to [p, b] tiles
        disD = sD.reshape(NB, P).T.copy()
        nd2D = nd2.reshape(NB, P).T.copy()
        # xt_own columns: x_n / s_r, slot-major
        xo = np.zeros((TPC, F_IN), np.float32)
        xo[valid] = x[nodes[valid]] * invD[valid][:, None]
        m = dict(common)
        m["xt_all"] = _build_xt1(x, dis, pl["row1"])
        m["xt_own"] = np.ascontiguousarray(xo.T.astype(bfloat16))
        m["invr"] = invD.reshape(1, TPC).astype(bfloat16)
        m["disD"] = disD.astype(np.float32)
        m["nd2D"] = nd2D.astype(np.float32)
        m["ix1"] = pl["ix1"]; m["dp1"] = pl["dp1"]
        m["ix2"] = pl["ix2"]; m["dp2"] = pl["dp2"]
        in_maps.append(m)

    res = None
    last_exc = None
    u1 = tuple(int(v) for v in plan["ucnt1"])
    u2 = tuple(int(v) for v in plan["ucnt2"])
    for g_try, nq_try in ((G, NQ), (16, 2), (4, 1)):
        key = (C1, C2, u1, u2, g_try, nq_try)
        try:
            if key not in _GRAPH_CACHE:
                _GRAPH_CACHE[key] = _build_graph(C1, C2, u1, u2, g_try, nq_try)
            res = run_bass_kernel_spmd(
                _GRAPH_CACHE[key], in_maps, core_ids=list(range(NCORE)))
            break
        except Exception as e:  # noqa: BLE001 - retry with safer gather config
            last_exc = e
            import time as _t
            _t.sleep(5)
    if res is None:
        raise last_exc
    kernel.last_result = res

    out_full = np.zeros((N, F_OUT), np.float32)
    for c in range(NCORE):
        o = res.results[c]["out"].transpose(1, 0, 2).reshape(TPC, F_OUT)
        valid = pi_inv[c] >= 0
        out_full[pi_inv[c][valid]] = o[valid]
    return out_full
